# revision 24
# baseline (speedup 1.0000x reference)
"""LSTM cell (batch 8192, input 512, hidden 512) on 8 Trainium2 NeuronCores.

Data-parallel over the batch dim: each core handles 1024 rows. Weights are
replicated. The host pre-transposes both matmul operands so the contraction
dim (fan_in = 1024) lands on SBUF partitions:

  gate.T[n, b] = sum_k W.T[k, n] * combined.T[k, b]     (matmul: lhsT.T @ rhs)

so the kernel computes everything in [hidden, batch] layout; gate biases
become per-partition vectors (free on the ACT activation op), and the host
transposes the outputs back after the gather.

Matmul operands are cast to bf16 on the host (f32 matmul on PE is 4x slower
per the cost model); accumulation is f32 in PSUM and the whole elementwise
tail (c_next = f*c_prev + i*c_tilde, h_next = o*tanh(c_next)) stays f32.
"""

import numpy as np

import concourse.bacc as bacc
import concourse.bass as bass
import concourse.mybir as mybir
from concourse import tile
from concourse.bass_utils import run_bass_kernel_spmd

N_CORES = 8
BATCH = 8192
B = BATCH // N_CORES  # 1024 batch rows per core
K = 1024              # fan_in = input_dim + hidden_dim
H = 512               # hidden dim
NG = 4                # gates: i, f, c, o
KT = K // 128         # 8 contraction tiles
HT = H // 128         # 4 hidden chunks per gate
BT = B // 512         # 2 batch halves (PSUM free-dim limit is 512 f32)

MM_DT = mybir.dt.bfloat16
F32 = mybir.dt.float32

_SIG = mybir.ActivationFunctionType.Sigmoid
_TANH = mybir.ActivationFunctionType.Tanh
# gate order within the concatenated weight: i, f, c, o
_GATE_FN = [_SIG, _SIG, _TANH, _SIG]


def _build():
    nc = bacc.Bacc(
        "TRN2",
        target_bir_lowering=False,
        debug=False,
        num_devices=N_CORES,
    )

    xhT = nc.dram_tensor("xhT", [K, B], MM_DT, kind="ExternalInput")
    # wTh column order is h-major: [h, g, p] -> col h*512 + g*128 + p, so the
    # h=0 slice of every k-tile is one contiguous 512-col strip. The ramp
    # then only streams the h=0 weights alongside the activations.
    wTh = nc.dram_tensor("wTh", [K, NG * H], MM_DT, kind="ExternalInput")
    bias2d = nc.dram_tensor("bias2d", [128, NG * HT], F32, kind="ExternalInput")
    c_prevT = nc.dram_tensor("c_prevT", [H, B], F32, kind="ExternalInput")
    h_nextT = nc.dram_tensor("h_nextT", [H, B], F32, kind="ExternalOutput")
    c_nextT = nc.dram_tensor("c_nextT", [H, B], F32, kind="ExternalOutput")

    with tile.TileContext(nc) as tc:
        with (
            tc.tile_pool(name="wts", bufs=1) as wpool,
            tc.tile_pool(name="acts", bufs=1) as apool,
            tc.tile_pool(name="cprev", bufs=1) as cpool,
            tc.tile_pool(name="gates", bufs=3) as gpool,
            tc.tile_pool(name="ew", bufs=3) as epool,
            tc.tile_pool(name="psum", bufs=1, space="PSUM") as pspool,
        ):
            # Stream inputs k-major so the first accumulation groups can
            # start before the full weight set has landed. c_prev tiles are
            # interleaved late: they are consumed only by the elementwise
            # tail, so they must not delay the weight/activation k-tiles.
            xh_tiles = []
            cp_tiles = [None] * HT

            def _load_cp(h):
                ct = cpool.tile([128, B], F32, tag=f"cp{h}", name=f"cp{h}")
                nc.sync.dma_start(ct[:], c_prevT[h * 128:(h + 1) * 128, :])
                cp_tiles[h] = ct

            wt_tiles = [[None] * HT for _ in range(KT)]  # [k][h] -> [128, 512]

            def _load_w(k, h):
                wt = wpool.tile([128, NG * 128], MM_DT, tag=f"w{k}_{h}", name=f"w{k}_{h}")
                nc.sync.dma_start(
                    wt[:], wTh[k * 128:(k + 1) * 128, h * 512:(h + 1) * 512]
                )
                wt_tiles[k][h] = wt

            # Ramp: per k-tile, activations + only the h=0 weight strip
            # (384KB) -- less than the PE's per-k appetite, so the h=0
            # matmuls run DMA-tight. Remaining h strips follow k-major,
            # finishing well before their compute phases.
            # Starter tiles: the very first matmul only needs gate 0's k=0
            # weight column block (32KB) and the first batch half of the k=0
            # activations (128KB), so carve those out as separate small DMAs
            # at the head of the queue.
            w00a = wpool.tile([128, 128], MM_DT, tag="w00a", name="w00a")
            nc.sync.dma_start(w00a[:], wTh[0:128, 0:128])
            xh0 = []
            for b2 in range(BT):
                xt = apool.tile([128, B // BT], MM_DT, tag=f"xh0_{b2}", name=f"xh0_{b2}")
                nc.sync.dma_start(
                    xt[:], xhT[0:128, b2 * 512:(b2 + 1) * 512]
                )
                xh0.append(xt)
            w00b = wpool.tile([128, 384], MM_DT, tag="w00b", name="w00b")
            nc.sync.dma_start(w00b[:], wTh[0:128, 128:512])

            bias_t = wpool.tile([128, NG * HT], F32, tag="bias", name="bias_t")
            nc.sync.dma_start(bias_t[:], bias2d[:])

            for k in range(1, KT):
                _load_w(k, 0)
                xt = apool.tile([128, B], MM_DT, tag=f"xh{k}", name=f"xh{k}")
                nc.sync.dma_start(xt[:], xhT[k * 128:(k + 1) * 128, :])
                xh_tiles.append(xt)
            for h in range(1, HT):
                _load_cp(h - 1)
                for k in range(KT):
                    _load_w(k, h)
            _load_cp(HT - 1)

            def _rhs(k, b2):
                if k == 0:
                    return xh0[b2][:]
                return xh_tiles[k - 1][:, b2 * 512:(b2 + 1) * 512]

            def _lhsT(k, h, g):
                if k == 0 and h == 0:
                    return w00a[:] if g == 0 else w00b[:, (g - 1) * 128:g * 128]
                return wt_tiles[k][h][:, g * 128:(g + 1) * 128]

            def _mk_psum(g, h, b2):
                return pspool.tile(
                    [128, 512], F32,
                    tag=f"ps{g}_{b2 % 2}", name=f"ps{g}_{h}_{b2}",
                )

            def _elementwise(h, b2, psum, chunks=1, dma_eng=None):
                """Activations + LSTM cell tail for one (h, b2) group.

                chunks>1 splits the free dim so the final group's serial
                ACT->DVE->ACT->DVE chain drains in smaller pieces.
                """
                dma_eng = dma_eng or nc.gpsimd
                hs = slice(h * 128, (h + 1) * 128)
                w = 512 // chunks
                def _act_gate(g, c):
                    t = gpool.tile(
                        [128, w], F32, tag=f"g{g}", name=f"g{g}_{h}_{b2}_{c}",
                    )
                    nc.scalar.activation(
                        t[:], psum[g][:, c * w:(c + 1) * w], _GATE_FN[g],
                        bias=bias_t[:, g * HT + h:g * HT + h + 1],
                    )
                    return t

                for c in range(chunks):
                    cs = slice(b2 * 512 + c * w, b2 * 512 + (c + 1) * w)
                    # i, f, c~ first; the whole c_next/tanh chain runs while
                    # the output gate's matmuls are still on the PE (gate-
                    # major issue order puts o last).
                    gi = _act_gate(0, c)
                    gf = _act_gate(1, c)
                    gc = _act_gate(2, c)

                    t1 = epool.tile([128, w], F32, tag="t1", name=f"t1_{h}_{b2}_{c}")
                    nc.vector.tensor_mul(t1[:], gi[:], gc[:])       # i * c~
                    t2 = epool.tile([128, w], F32, tag="t2", name=f"t2_{h}_{b2}_{c}")
                    nc.vector.tensor_mul(t2[:], gf[:], cp_tiles[h][:, cs])
                    cn = epool.tile([128, w], F32, tag="cn", name=f"cn_{h}_{b2}_{c}")
                    nc.vector.tensor_add(cn[:], t1[:], t2[:])
                    dma_eng.dma_start(c_nextT[hs, cs], cn[:])

                    th = epool.tile([128, w], F32, tag="th", name=f"th_{h}_{b2}_{c}")
                    nc.scalar.activation(th[:], cn[:], _TANH)

                    go = _act_gate(3, c)
                    hn = epool.tile([128, w], F32, tag="hn", name=f"hn_{h}_{b2}_{c}")
                    nc.vector.tensor_mul(hn[:], go[:], th[:])
                    dma_eng.dma_start(h_nextT[hs, cs], hn[:])

            # h=0 rides the input-DMA ramp: every group needs all 8 k-tiles,
            # so widen to all 8 PSUM banks (4 gates x 2 batch halves) and
            # issue k-major -- the PE consumes each k-tile pair 8 matmuls at
            # a time, right as it lands.
            psum0 = {b2: [_mk_psum(g, 0, b2) for g in range(NG)] for b2 in range(BT)}
            for k in range(KT):
                for g in range(NG):
                    for b2 in range(BT):
                        nc.tensor.matmul(
                            psum0[b2][g][:],
                            _lhsT(k, 0, g),
                            _rhs(k, b2),
                            start=(k == 0),
                            stop=(k == KT - 1),
                        )
            for b2 in range(BT):
                _elementwise(0, b2, psum0[b2])

            # h>=1: inputs are resident; per-(h,b2) 4-bank groups with b2
            # parity alternating between the two bank sets, so each set's
            # ACT drain overlaps the other's matmuls.
            for h in range(1, HT):
                for b2 in range(BT):
                    psum = [_mk_psum(g, h, b2) for g in range(NG)]
                    # gate-major, output gate (g=3) last: everything except
                    # ACT(o) and h=o*tanh(c) drains while o's matmuls run.
                    for g in range(NG):
                        for k in range(KT):
                            nc.tensor.matmul(
                                psum[g][:],
                                _lhsT(k, h, g),
                                _rhs(k, b2),
                                start=(k == 0),
                                stop=(k == KT - 1),
                            )
                    last = (h == HT - 1 and b2 == BT - 1)
                    _elementwise(
                        h, b2, psum,
                        dma_eng=nc.sync if last else None,
                    )

    nc.compile()
    return nc


_NC_CACHE = None
_LAST_IN_MAPS = None


def kernel(x, h_prev, c_prev, W_i, b_i, W_f, b_f, W_c, b_c, W_o, b_o):
    global _NC_CACHE, _LAST_IN_MAPS
    if _NC_CACHE is None:
        _NC_CACHE = _build()
    nc = _NC_CACHE

    np_bf16 = mybir.dt.np(MM_DT)

    combT = np.concatenate([x, h_prev], axis=1).T          # (K, BATCH) f32
    combT = combT.astype(np_bf16)
    wT = np.concatenate([W_i, W_f, W_c, W_o], axis=0).T    # (K, 4H): col g*H+h*128+p
    # h-major column order: col h*512 + g*128 + p  (see _build)
    wTh = np.ascontiguousarray(
        wT.reshape(K, NG, HT, 128).transpose(0, 2, 1, 3).reshape(K, NG * H)
    ).astype(np_bf16)
    bias2d = np.ascontiguousarray(
        np.concatenate([b_i, b_f, b_c, b_o]).reshape(NG * HT, 128).T
    ).astype(np.float32)                                   # (128, 16)
    c_prevT = c_prev.T                                     # (H, BATCH)

    in_maps = []
    for j in range(N_CORES):
        cols = slice(j * B, (j + 1) * B)
        in_maps.append({
            "xhT": np.ascontiguousarray(combT[:, cols]),
            "wTh": wTh,
            "bias2d": bias2d,
            "c_prevT": np.ascontiguousarray(c_prevT[:, cols], dtype=np.float32),
        })

    _LAST_IN_MAPS = in_maps
    res = run_bass_kernel_spmd(nc, in_maps, core_ids=list(range(N_CORES)))

    h_next = np.concatenate([r["h_nextT"].T for r in res.results], axis=0)
    c_next = np.concatenate([r["c_nextT"].T for r in res.results], axis=0)
    return (h_next.astype(np.float32), c_next.astype(np.float32))
